# revision 1
# baseline (speedup 1.0000x reference)
# Multi-head attention (B=2, S=2048, D=1024, H=16) on 8 TRN2 NeuronCores.
#
# Sharding (hardcoded): core c in [0..8) handles batch b = c//4 and head
# group g = c%4 (4 heads = 256 output features of wq/wk/wv, 256 input rows
# of wo). Each core computes a partial output projection [S, D]; the host
# sums the 4 partials per batch and adds wo_bias (row-parallel unshard).
#
# Device-side layout choices:
#   - activations enter transposed ([D, S]) so every matmul contracts over
#     the partition axis with no on-device transposes;
#   - scores are computed transposed (S^T[k, q]) so softmax(P) feeds the
#     P@V matmul directly (contraction over k on partitions);
#   - the softmax denominator comes free as an extra ones-column appended
#     to each head's V block (output row 64 of the PV accumulation);
#   - matmuls run in float32r (full-rate fp32 path for moving dim >= 256);
#     P/V/out-proj run in bf16.
import functools
import sys

import numpy as np

try:
    import concourse  # noqa: F401
except ImportError:  # harness env without the default path
    sys.path.insert(0, "/opt/trn_rl_repo")
    sys.path.insert(0, "/opt/pypackages")

import ml_dtypes

BF16 = ml_dtypes.bfloat16

B, S, D, H = 2, 2048, 1024, 16
HD = D // H          # 64
NCORES = 8
GH = 4               # head groups (tensor-parallel)
HPG = H // GH        # heads per group = 4
DG = D // GH         # features per group = 256
P = 128              # partitions
TDIN = D // P        # 8 din tiles
SC = 4               # s-chunks of 512 for projections
CW = S // SC         # 512
QC = 2               # q-chunks of 1024 for attention
QW = S // QC         # 1024
KT = S // P          # 16 k tiles
NT2 = DG // P        # 2 dout tiles per group


def build_graph():
    """Build the SPMD Bass graph (identical on all 8 cores)."""
    from contextlib import ExitStack

    from concourse import bacc, mybir, tile

    f32 = mybir.dt.float32
    f32r = mybir.dt.float32r
    bf16 = mybir.dt.bfloat16
    EXP = mybir.ActivationFunctionType.Exp

    nc = bacc.Bacc(
        "TRN2", target_bir_lowering=False, debug=False, num_devices=NCORES
    )

    xq = nc.dram_tensor("xq_t", (P, TDIN, S), bf16, kind="ExternalInput")
    xk = nc.dram_tensor("xk_t", (P, TDIN, S), bf16, kind="ExternalInput")
    xv = nc.dram_tensor("xv_t", (P, TDIN, S), bf16, kind="ExternalInput")
    mk = nc.dram_tensor("mask_t", (S, S), bf16, kind="ExternalInput")
    wq = nc.dram_tensor("wq", (P, TDIN, DG), bf16, kind="ExternalInput")
    wk = nc.dram_tensor("wk", (P, TDIN, DG), bf16, kind="ExternalInput")
    wv = nc.dram_tensor("wv", (P, TDIN, DG), bf16, kind="ExternalInput")
    # wo pre-arranged host-side to [64, HPG, D] (j, h, n) so each head's
    # 64 rows sit on partitions 0..63.
    wo = nc.dram_tensor("wo", (HD, HPG, D), bf16, kind="ExternalInput")
    qb = nc.dram_tensor("qb", (1, DG), bf16, kind="ExternalInput")
    kb = nc.dram_tensor("kb", (1, DG), bf16, kind="ExternalInput")
    vb = nc.dram_tensor("vb", (1, DG), bf16, kind="ExternalInput")
    out = nc.dram_tensor("out", (S, D), bf16, kind="ExternalOutput")

    with tile.TileContext(nc) as tc, ExitStack() as ctx:
        wpool = ctx.enter_context(tc.tile_pool(name="wpool", bufs=1))
        cpool = ctx.enter_context(tc.tile_pool(name="cpool", bufs=1))
        qkpool = ctx.enter_context(tc.tile_pool(name="qk", bufs=1))
        vpool = ctx.enter_context(tc.tile_pool(name="vsb", bufs=1))
        mpool = ctx.enter_context(tc.tile_pool(name="msk", bufs=1))
        ppool = ctx.enter_context(tc.tile_pool(name="ptile", bufs=3))
        spool = ctx.enter_context(tc.tile_pool(name="small", bufs=2))
        dpool = ctx.enter_context(tc.tile_pool(name="dscr", bufs=2, space="DRAM"))
        bigps = ctx.enter_context(tc.tile_pool(name="bigps", bufs=3, space="PSUM"))
        ops_pool = ctx.enter_context(tc.tile_pool(name="ops", bufs=1, space="PSUM"))

        # ---- persistent SBUF tensors -------------------------------------
        wq_sb = wpool.tile([P, TDIN, DG], bf16)
        wk_sb = wpool.tile([P, TDIN, DG], bf16)
        wv_sb = wpool.tile([P, TDIN, DG], bf16)
        for wsb_, wdr_ in ((wq_sb, wq), (wk_sb, wk), (wv_sb, wv)):
            for th_ in range(2):
                nc.sync.dma_start(
                    wsb_[:, th_ * 4 : (th_ + 1) * 4, :],
                    wdr_.ap()[:, th_ * 4 : (th_ + 1) * 4, :],
                )
        wo_sb = wpool.tile([HD, HPG, D], bf16)
        nc.sync.dma_start(wo_sb[:], wo.ap())
        qb_sb = cpool.tile([1, DG], bf16)
        kb_sb = cpool.tile([1, DG], bf16)
        vb_sb = cpool.tile([1, DG], bf16)
        nc.sync.dma_start(qb_sb[:], qb.ap())
        nc.sync.dma_start(kb_sb[:], kb.ap())
        nc.sync.dma_start(vb_sb[:], vb.ap())
        # ones: row 0 used as [1, CW] rhs / [1, P] lhsT at partition 0;
        # row 64 used as [1, HD] lhsT at partition 64 (denominator bcast).
        ones2 = cpool.tile([1, CW], bf16)
        nc.vector.memset(ones2[:], 1.0)

        qT_sb = qkpool.tile([P, NT2, S], bf16)   # q projection, transposed
        kT_sb = qkpool.tile([P, NT2, S], bf16)
        # v blocks: per k-tile, per head: [v(64) | ones] -> 65 cols
        v_sb = vpool.tile([P, KT, HPG * (HD + 1)], bf16)
        nc.vector.memset(
            v_sb[:].rearrange("p s (h x) -> p s h x", h=HPG)[:, :, :, HD : HD + 1],
            1.0,
        )
        # ---- projections -------------------------------------------------
        # q, k: out qT[dout, s] = wq^T(stationary) x q^T(moving) + bias
        xpool_cm = tc.tile_pool(name="xin", bufs=2)
        xpool = xpool_cm.__enter__()
        NCH = S // 1024
        for xdram, wsb, bias_sb, dest in (
            (xq, wq_sb, qb_sb, qT_sb),
            (xk, wk_sb, kb_sb, kT_sb),
        ):
            for sc in range(NCH):
                xch = xpool.tile([P, TDIN, 1024], bf16, tag="xch")
                for th_ in range(4):
                    nc.sync.dma_start(
                        xch[:, th_ * 2 : (th_ + 1) * 2, :],
                        xdram.ap()[
                            :, th_ * 2 : (th_ + 1) * 2, sc * 1024 : (sc + 1) * 1024
                        ],
                    )
                for half in range(2):
                    s0 = sc * 1024 + half * 512
                    for dt in range(NT2):
                        ps = bigps.tile(
                            [P, CW], f32, tag="ps", name=f"pj_{sc}_{half}_{dt}"
                        )
                        for ktl in range(TDIN):
                            nc.tensor.matmul(
                                ps[:],
                                lhsT=wsb[:, ktl, dt * P : (dt + 1) * P],
                                rhs=xch[:, ktl, half * 512 : (half + 1) * 512],
                                start=(ktl == 0),
                                stop=False,
                            )
                        nc.tensor.matmul(
                            ps[:],
                            lhsT=bias_sb[0:1, dt * P : (dt + 1) * P],
                            rhs=ones2[0:1, :],
                            start=False,
                            stop=True,
                        )
                        nc.vector.tensor_copy(
                            dest[:, dt, s0 : s0 + 512], ps[:]
                        )
        # v: natural layout [s, dout] + bias, drained per-head with ones col
        for sc in range(NCH):
            xch = xpool.tile([P, TDIN, 1024], bf16, tag="xch")
            for th_ in range(4):
                nc.sync.dma_start(
                    xch[:, th_ * 2 : (th_ + 1) * 2, :],
                    xv.ap()[
                        :, th_ * 2 : (th_ + 1) * 2, sc * 1024 : (sc + 1) * 1024
                    ],
                )
            for m in range(1024 // P):
                st = sc * (1024 // P) + m
                ps = bigps.tile([P, DG], f32, tag="ps", name=f"pv_{sc}_{m}")
                for ktl in range(TDIN):
                    nc.tensor.matmul(
                        ps[:],
                        lhsT=xch[:, ktl, m * P : (m + 1) * P],
                        rhs=wv_sb[:, ktl, :],
                        start=(ktl == 0),
                        stop=False,
                    )
                nc.tensor.matmul(
                    ps[:],
                    lhsT=ones2[0:1, 0:P],
                    rhs=vb_sb[:],
                    start=False,
                    stop=True,
                )
                nc.vector.tensor_copy(
                    v_sb[:, st, :].rearrange("p (h x) -> p h x", h=HPG)[
                        :, :, 0:HD
                    ],
                    ps[:].rearrange("p (h x) -> p h x", h=HPG),
                )
        xpool_cm.__exit__(None, None, None)

        # mask load issued after projection DMAs so it doesn't hog queues
        mask_sb = mpool.tile([P, KT, S], bf16)
        mk_r = mk.ap().rearrange("(t p) q -> p t q", p=P)
        for kt in range(KT):
            nc.sync.dma_start(mask_sb[:, kt, :], mk_r[:, kt, :])

        # ---- attention ---------------------------------------------------
        # One head at a time; score psum triple-buffered so the PE can run
        # up to 3 k-tiles ahead of the exp/mask/PV chain.
        opool_sb = ctx.enter_context(tc.tile_pool(name="otn", bufs=1))
        otn_sb = opool_sb.tile([HD, HPG, S], bf16)

        def emit_outproj(st):
            osb2 = ppool.tile([P, D], bf16, tag="outsb", name=f"outsb_{st}")
            for nch in range(2):
                op_ps = bigps.tile(
                    [P, 512], f32, tag="ps", name=f"ops2_{st}_{nch}"
                )
                for h_ in range(HPG):
                    nc.tensor.matmul(
                        op_ps[:],
                        lhsT=otn_sb[:, h_, st * P : (st + 1) * P],
                        rhs=wo_sb[:, h_, nch * 512 : (nch + 1) * 512],
                        start=(h_ == 0),
                        stop=(h_ == HPG - 1),
                    )
                nc.vector.tensor_copy(
                    osb2[:, nch * 512 : (nch + 1) * 512], op_ps[:]
                )
            nc.sync.dma_start(out.ap()[st * P : (st + 1) * P, :], osb2[:])

        pending_st = []
        for qc in range(QC):
            for h in range(HPG):
                t, po = h // 2, (h % 2) * HD
                o_ps = ops_pool.tile(
                    [HD + 1, QW], f32, tag="ops", name=f"ops_{qc}_{h}"
                )
                for kt in range(KT):
                    s_ps = bigps.tile(
                        [P, QW], f32, tag="ps", name=f"sps_{qc}_{h}_{kt}"
                    )
                    for hf in range(2):
                        nc.tensor.matmul(
                            s_ps[:, hf * 512 : (hf + 1) * 512],
                            lhsT=kT_sb[po : po + HD, t, kt * P : (kt + 1) * P],
                            rhs=qT_sb[
                                po : po + HD,
                                t,
                                qc * QW + hf * 512 : qc * QW + (hf + 1) * 512,
                            ],
                            start=True,
                            stop=True,
                        )
                    pt = ppool.tile(
                        [P, QW], bf16, tag="p", name=f"pt_{qc}_{h}_{kt}"
                    )
                    nc.scalar.activation(pt[:], s_ps[:], EXP, scale=0.125)
                    nc.vector.tensor_mul(
                        pt[:], pt[:], mask_sb[:, kt, qc * QW : (qc + 1) * QW]
                    )
                    for hf in range(2):
                        nc.tensor.matmul(
                            o_ps[:, hf * 512 : (hf + 1) * 512],
                            lhsT=v_sb[:, kt, h * 65 : (h + 1) * 65],
                            rhs=pt[:, hf * 512 : (hf + 1) * 512],
                            start=(kt == 0),
                            stop=(kt == KT - 1),
                        )
                # softmax normalization (no PE): approx-recip of the
                # denominator row, DRAM-bounce broadcast, one TT multiply.
                rec65 = spool.tile([HD + 1, QW], f32, tag="rec")
                nc.vector.reciprocal_approx_fast(out=rec65[:], in_=o_ps[:])
                osb = spool.tile([HD, QW], f32, tag="osb")
                nc.vector.tensor_copy(osb[:], o_ps[0:HD, :])
                scr = dpool.tile([1, QW], f32, tag="scr", name=f"scr_{qc}_{h}")
                nc.sync.dma_start(scr[:], rec65[HD : HD + 1, :])
                rb = spool.tile([HD, QW], f32, tag="rb")
                nc.sync.dma_start(rb[:], scr[:].to_broadcast((HD, QW)))
                nc.vector.tensor_mul(
                    otn_sb[:, h, qc * QW : (qc + 1) * QW], osb[:], rb[:]
                )

            pending_st.extend(range(qc * (QW // P), (qc + 1) * (QW // P)))

        for st in pending_st:
            emit_outproj(st)

    nc.compile()
    return nc


@functools.lru_cache(maxsize=1)
def _graph():
    return build_graph()


def make_in_maps(
    query, key, value, mask,
    wq_kernel, wq_bias, wk_kernel, wk_bias,
    wv_kernel, wv_bias, wo_kernel, wo_bias,
):
    q = np.asarray(query, np.float32)
    k = np.asarray(key, np.float32)
    v = np.asarray(value, np.float32)
    mask = np.asarray(mask)
    wqk = np.asarray(wq_kernel, np.float32)
    wkk = np.asarray(wk_kernel, np.float32)
    wvk = np.asarray(wv_kernel, np.float32)
    wok = np.asarray(wo_kernel, np.float32)

    def tile_x(a):  # [S, D] -> [P, TDIN, S] pre-tiled transpose
        return np.ascontiguousarray(
            a.T.reshape(TDIN, P, S).transpose(1, 0, 2)
        ).astype(BF16)

    xt = [[tile_x(x[b]) for x in (q, k, v)] for b in range(B)]
    mt = [
        np.ascontiguousarray(mask[b].T.astype(np.float32)).astype(BF16)
        for b in range(B)
    ]
    in_maps = []
    for c in range(NCORES):
        b, g = divmod(c, GH)
        cs = slice(g * DG, (g + 1) * DG)
        wo_arr = np.ascontiguousarray(
            wok[cs, :].reshape(HPG, HD, D).transpose(1, 0, 2)
        ).astype(BF16)
        in_maps.append(
            {
                "xq_t": xt[b][0],
                "xk_t": xt[b][1],
                "xv_t": xt[b][2],
                "mask_t": mt[b],
                "wq": np.ascontiguousarray(wqk[:, cs].reshape(TDIN, P, DG).transpose(1, 0, 2)).astype(BF16),
                "wk": np.ascontiguousarray(wkk[:, cs].reshape(TDIN, P, DG).transpose(1, 0, 2)).astype(BF16),
                "wv": np.ascontiguousarray(wvk[:, cs].reshape(TDIN, P, DG).transpose(1, 0, 2)).astype(BF16),
                "wo": wo_arr,
                "qb": np.asarray(wq_bias, np.float32)[cs].reshape(1, DG).astype(BF16),
                "kb": np.asarray(wk_bias, np.float32)[cs].reshape(1, DG).astype(BF16),
                "vb": np.asarray(wv_bias, np.float32)[cs].reshape(1, DG).astype(BF16),
            }
        )
    return in_maps


def combine_outputs(results, wo_bias):
    outs = np.stack([np.asarray(r["out"], np.float32) for r in results])
    full = outs.reshape(B, GH, S, D).sum(axis=1)
    return (full + np.asarray(wo_bias, np.float32)[None, None, :]).astype(
        np.float32
    )


def kernel(**inputs):
    from concourse import bass_utils

    nc = _graph()
    in_maps = make_in_maps(**inputs)
    res = bass_utils.run_bass_kernel_spmd(
        nc, in_maps, core_ids=list(range(NCORES))
    )
    return combine_outputs(res.results, inputs["wo_bias"])



# revision 17
# speedup vs baseline: 1.1303x; 1.1303x over previous
# Multi-head attention (B=2, S=2048, D=1024, H=16) on 8 TRN2 NeuronCores.
#
# Sharding (hardcoded): core c in [0..8) handles batch b = c//4 and head
# group g = c%4 (4 heads = 256 output features of wq/wk/wv, 256 input rows
# of wo). Each core computes a partial output projection [S, D]; the host
# sums the 4 partials per batch and adds wo_bias (row-parallel unshard).
#
# v2 design notes (engines balanced around the Scalar exp floor, ~147us):
#   - all matmuls bf16 (fp8 fails the 2e-2 tolerance: random-sign dots keep
#     the per-element quant error, and scores enter exp).
#   - activations enter transposed ([D, S]); scores computed transposed
#     (S^T[k, q]) so softmax(P) feeds P@V directly; denominator comes free
#     as a ones-column appended to each head's V block.
#   - output projection contracts K=128 by packing head PAIRS into
#     otn2[128, t, S]; odd heads reach partitions 64..127 via a small
#     SBUF->SBUF DMA (engines cannot shift partitions).
#   - emission is pipelined so exp starts ~12us in: Q-sc0 -> K-t0 ->
#     scores-h0 ... with V-projection tiles and K-t1 interleaved between
#     score tiles to fill the PE while exp runs. h0's PV is deferred (its
#     P tiles stay resident) until V exists; o_ps is copied to SBUF right
#     after PV so the single PSUM accumulator frees quickly.
#   - Q/K biases fused into drains (scalar Identity+bias during idle proj
#     phase); late drains and outproj drains go to GpSimd so the Vector
#     engine mostly runs mask-mult + normalization.
import functools
import sys

import numpy as np

try:
    import concourse  # noqa: F401
except ImportError:  # harness env without the default path
    sys.path.insert(0, "/opt/trn_rl_repo")
    sys.path.insert(0, "/opt/pypackages")

import ml_dtypes

BF16 = ml_dtypes.bfloat16

B, S, D, H = 2, 2048, 1024, 16
HD = D // H          # 64
NCORES = 8
GH = 4               # head groups (tensor-parallel)
HPG = H // GH        # heads per group = 4
DG = D // GH         # features per group = 256
P = 128              # partitions
TDIN = D // P        # 8 din tiles
QC = 2               # q-chunks of 1024 for attention
QW = S // QC         # 1024
KT = S // P          # 16 k tiles
NT2 = DG // P        # 2 dout tiles per group


def build_graph():
    """Build the SPMD Bass graph (identical on all 8 cores)."""
    from contextlib import ExitStack

    from concourse import bacc, mybir, tile

    f32 = mybir.dt.float32
    bf16 = mybir.dt.bfloat16
    EXP = mybir.ActivationFunctionType.Exp
    IDENT = mybir.ActivationFunctionType.Identity

    nc = bacc.Bacc(
        "TRN2", target_bir_lowering=False, debug=False, num_devices=NCORES
    )

    xq = nc.dram_tensor("xq_t", (P, TDIN, S), bf16, kind="ExternalInput")
    xk = nc.dram_tensor("xk_t", (P, TDIN, S), bf16, kind="ExternalInput")
    xv = nc.dram_tensor("xv_t", (P, TDIN, S), bf16, kind="ExternalInput")
    mk = nc.dram_tensor("mask_t", (S, S), bf16, kind="ExternalInput")
    wq = nc.dram_tensor("wq", (P, TDIN, DG), bf16, kind="ExternalInput")
    wk = nc.dram_tensor("wk", (P, TDIN, DG), bf16, kind="ExternalInput")
    wv = nc.dram_tensor("wv", (P, TDIN, DG), bf16, kind="ExternalInput")
    # wo pre-arranged host-side to [128, NT2, D]: partition p = (h%2)*64+hd,
    # tile t = h//2 (head pair), so outproj contracts K=128 over 2 heads.
    wo = nc.dram_tensor("wo", (P, NT2, D), bf16, kind="ExternalInput")
    # q/k biases as per-partition columns [128, NT2]; v bias as a row.
    qb = nc.dram_tensor("qb", (P, NT2), f32, kind="ExternalInput")
    kb = nc.dram_tensor("kb", (P, NT2), f32, kind="ExternalInput")
    vb = nc.dram_tensor("vb", (1, DG), bf16, kind="ExternalInput")
    out = nc.dram_tensor("out", (S, D), bf16, kind="ExternalOutput")

    with tile.TileContext(nc) as tc, ExitStack() as ctx:
        wpool = ctx.enter_context(tc.tile_pool(name="wpool", bufs=1))
        cpool = ctx.enter_context(tc.tile_pool(name="cpool", bufs=1))
        qkpool = ctx.enter_context(tc.tile_pool(name="qk", bufs=1))
        vpool = ctx.enter_context(tc.tile_pool(name="vsb", bufs=1))
        mpool = ctx.enter_context(tc.tile_pool(name="msk", bufs=1))
        opool = ctx.enter_context(tc.tile_pool(name="otn", bufs=1))
        xstage = ctx.enter_context(tc.tile_pool(name="xin", bufs=2))
        ptpool = ctx.enter_context(tc.tile_pool(name="pt", bufs=16))
        npool = ctx.enter_context(tc.tile_pool(name="nrm", bufs=1))
        ospool = ctx.enter_context(tc.tile_pool(name="osbp", bufs=1))
        dpool = ctx.enter_context(tc.tile_pool(name="dscr", bufs=2, space="DRAM"))
        # PSUM: scores [128,1024] x2 (4 banks) + o_ps [65,1024] x1 (2 banks)
        # + proj/outproj [128,512] x2 (2 banks) = 8 banks exactly.
        spspool = ctx.enter_context(tc.tile_pool(name="sps", bufs=2, space="PSUM"))
        opspool = ctx.enter_context(tc.tile_pool(name="ops", bufs=1, space="PSUM"))
        pjpool = ctx.enter_context(tc.tile_pool(name="pjps", bufs=2, space="PSUM"))

        # ---- persistent SBUF tensors & input DMAs ------------------------
        # Per-tensor staging pools (512-col chunks) so no staging DMA ever
        # waits on a reader that sits later in the PE stream (deadlock).
        def stage_x(xdr, c, tag, name):
            xt = xstage.tile([P, TDIN, 512], bf16, tag=tag, name=name)
            for th_ in range(4):
                nc.sync.dma_start(
                    xt[:, th_ * 2 : (th_ + 1) * 2, :],
                    xdr.ap()[:, th_ * 2 : (th_ + 1) * 2, c * 512 : (c + 1) * 512],
                )
            return xt

        xq_c = [stage_x(xq, 0, "xq", "xq_c0"), stage_x(xq, 1, "xq", "xq_c1")]
        wq_sb = wpool.tile([P, TDIN, DG], bf16)
        wk_sb = wpool.tile([P, TDIN, DG], bf16)
        for wsb_, wdr_ in ((wq_sb, wq), (wk_sb, wk)):
            for th_ in range(2):
                nc.sync.dma_start(
                    wsb_[:, th_ * 4 : (th_ + 1) * 4, :],
                    wdr_.ap()[:, th_ * 4 : (th_ + 1) * 4, :],
                )
        qb_sb = cpool.tile([P, NT2], f32)
        kb_sb = cpool.tile([P, NT2], f32)
        nc.sync.dma_start(qb_sb[:], qb.ap())
        nc.sync.dma_start(kb_sb[:], kb.ap())

        xk_c = [stage_x(xk, 0, "xk", "xk_c0"), stage_x(xk, 1, "xk", "xk_c1")]

        # mask per-kt (kt0 first: needed ~15us in)
        mask_sb = mpool.tile([P, KT, S], bf16)
        mk_r = mk.ap().rearrange("(t p) q -> p t q", p=P)
        for kt in range(KT):
            nc.sync.dma_start(mask_sb[:, kt, :], mk_r[:, kt, :])

        wv_sb = wpool.tile([P, TDIN, DG], bf16)
        for th_ in range(2):
            nc.sync.dma_start(
                wv_sb[:, th_ * 4 : (th_ + 1) * 4, :],
                wv.ap()[:, th_ * 4 : (th_ + 1) * 4, :],
            )
        wo_sb = wpool.tile([P, NT2, D], bf16)
        nc.sync.dma_start(wo_sb[:], wo.ap())
        vb_sb = cpool.tile([1, DG], bf16)
        nc.sync.dma_start(vb_sb[:], vb.ap())
        ones2 = cpool.tile([1, P], bf16)
        nc.vector.memset(ones2[:], 1.0)

        qT_sb = qkpool.tile([P, NT2, S], bf16)   # q projection, transposed
        kT_sb = qkpool.tile([P, NT2, S], bf16)
        # v blocks: per k-tile, per head: [v(64) | ones] -> 65 cols
        v_sb = vpool.tile([P, KT, HPG * (HD + 1)], bf16)
        nc.vector.memset(
            v_sb[:].rearrange("p s (h x) -> p s h x", h=HPG)[:, :, :, HD : HD + 1],
            1.0,
        )
        # packed normalized attention output: partition (h%2)*64+hd, tile h//2
        otn2 = opool.tile([P, NT2, S], bf16)

        # ---- projection helpers ------------------------------------------
        def emit_qk_chunk(xsb, xoff, wsb, bias_sb, dest, dt, s0, drain_eng):
            """Project one [128 dout, 512 s] tile: 8 acc matmuls + drain."""
            ps = pjpool.tile([P, 512], f32, tag="pj", name=f"pj_{dt}_{s0}_{drain_eng}")
            for ktl in range(TDIN):
                nc.tensor.matmul(
                    ps[:],
                    lhsT=wsb[:, ktl, dt * P : (dt + 1) * P],
                    rhs=xsb[:, ktl, s0 - xoff : s0 - xoff + 512],
                    start=(ktl == 0),
                    stop=(ktl == TDIN - 1),
                )
            if drain_eng == "scalar":
                nc.scalar.activation(
                    dest[:, dt, s0 : s0 + 512], ps[:], IDENT,
                    bias=bias_sb[:, dt : dt + 1], scale=1.0,
                )
            else:
                nc.vector.tensor_scalar_add(
                    dest[:, dt, s0 : s0 + 512], ps[:], bias_sb[:, dt : dt + 1]
                )

        def emit_v_mtile(xsb, xoff, st):
            """V projection for s-tile st ([128 s, 256 dout] + bias + ones)."""
            ps = pjpool.tile([P, 512], f32, tag="pj", name=f"pv_{st}")
            for ktl in range(TDIN):
                nc.tensor.matmul(
                    ps[:, 0:DG],
                    lhsT=xsb[:, ktl, st * P - xoff : (st + 1) * P - xoff],
                    rhs=wv_sb[:, ktl, :],
                    start=(ktl == 0),
                    stop=False,
                )
            nc.tensor.matmul(
                ps[:, 0:DG],
                lhsT=ones2[0:1, :],
                rhs=vb_sb[:],
                start=False,
                stop=True,
            )
            dst = v_sb[:, st, :].rearrange("p (h x) -> p h x", h=HPG)[:, :, 0:HD]
            src = ps[:, 0:DG].rearrange("p (h x) -> p h x", h=HPG)
            nc.vector.tensor_copy(dst, src)

        # ---- attention helpers -------------------------------------------
        def emit_scores_kt(qc, h, kt):
            """scores (2 matmuls) -> exp -> mask-mult; returns P tile."""
            t, po = h // 2, (h % 2) * HD
            s_ps = spspool.tile([P, QW], f32, tag="sps", name=f"sps_{qc}_{h}_{kt}")
            for hf in range(2):
                nc.tensor.matmul(
                    s_ps[:, hf * 512 : (hf + 1) * 512],
                    lhsT=kT_sb[po : po + HD, t, kt * P : (kt + 1) * P],
                    rhs=qT_sb[
                        po : po + HD, t,
                        qc * QW + hf * 512 : qc * QW + (hf + 1) * 512,
                    ],
                    start=True,
                    stop=True,
                )
            pt = ptpool.tile([P, QW], bf16, tag="p", name=f"pt_{qc}_{h}_{kt}")
            nc.scalar.activation(pt[:], s_ps[:], EXP, scale=0.125)
            meng = nc.gpsimd if kt in (4, 9, 14) else nc.vector
            meng.tensor_mul(
                pt[:], pt[:], mask_sb[:, kt, qc * QW : (qc + 1) * QW]
            )
            return pt

        def emit_pv_kt(h, kt, pt, o_ps):
            for hf in range(2):
                nc.tensor.matmul(
                    o_ps[:, hf * 512 : (hf + 1) * 512],
                    lhsT=v_sb[:, kt, h * 65 : (h + 1) * 65],
                    rhs=pt[:, hf * 512 : (hf + 1) * 512],
                    start=(kt == 0),
                    stop=(kt == KT - 1),
                )

        def emit_osb_copy(qc, h, o_ps):
            """Reciprocal (needs f32) + SBUF copy so the PSUM frees quickly."""
            rec65 = npool.tile([HD + 1, QW], f32, tag="rec")
            nc.vector.reciprocal_approx_fast(out=rec65[:], in_=o_ps[:])
            osb = ospool.tile([HD + 1, QW], bf16, tag="osb", name=f"osb_{qc}_{h}")
            nc.vector.tensor_copy(osb[:], o_ps[:])
            return osb, rec65

        def emit_norm(qc, h, osb, rec65):
            """softmax normalization; writes otn2 (odd heads via DMA)."""
            t = h // 2
            scr = dpool.tile([1, QW], f32, tag="scr", name=f"scr_{qc}_{h}")
            nc.sync.dma_start(scr[:], rec65[HD : HD + 1, :])
            rb = npool.tile([HD, QW], f32, tag="rb")
            nc.sync.dma_start(rb[:], scr[:].to_broadcast((HD, QW)))
            if h % 2 == 0:
                nc.vector.tensor_mul(
                    otn2[0:HD, t, qc * QW : (qc + 1) * QW], osb[0:HD, :], rb[:]
                )
            else:
                nc.vector.tensor_mul(osb[0:HD, :], osb[0:HD, :], rb[:])
                nc.sync.dma_start(
                    otn2[HD:P, t, qc * QW : (qc + 1) * QW], osb[0:HD, :]
                )

        def emit_outproj(st):
            osb2 = ospool.tile([P, D], bf16, tag="outsb", name=f"outsb_{st}",
                               bufs=2)
            for nch in range(2):
                op_ps = pjpool.tile(
                    [P, 512], f32, tag="pj", name=f"ops2_{st}_{nch}"
                )
                for t in range(NT2):
                    nc.tensor.matmul(
                        op_ps[:],
                        lhsT=otn2[:, t, st * P : (st + 1) * P],
                        rhs=wo_sb[:, t, nch * 512 : (nch + 1) * 512],
                        start=(t == 0),
                        stop=(t == NT2 - 1),
                    )
                nc.vector.tensor_copy(osb2[:, nch * 512 : (nch + 1) * 512], op_ps[:])
            nc.sync.dma_start(out.ap()[st * P : (st + 1) * P, :], osb2[:])

        # ---- emission schedule -------------------------------------------
        # Uniform deferred-by-one pipeline: head-sequence n = (qc, h); each
        # head's 16 score-kt "slots" carry fillers: PV of the PREVIOUS head
        # (2 kt-pairs per slot over slots 0-7), V-projection tiles, K-t1 /
        # Q-sc1 chunks (re-staged with prefetch), and outproj tiles.
        seq = [(qc, h) for qc in range(QC) for h in range(HPG)]

        # prefix: all Q-qc0 + K-t0 chunks (xk c2/c3 staged rolling).
        for dt in range(NT2):
            for c in range(2):
                emit_qk_chunk(xq_c[c], c * 512, wq_sb, qb_sb, qT_sb, dt,
                              c * 512, "scalar")
        for c in range(4):
            if c >= 2:
                xk_c.append(stage_x(xk, c, "xk", f"xk_c{c}"))
            emit_qk_chunk(xk_c[c], c * 512, wk_sb, kb_sb, kT_sb, 0,
                          c * 512, "scalar")

        xv_c = [stage_x(xv, 0, "xv", "xv_c0"), stage_x(xv, 1, "xv", "xv_c1")]

        def v_tile(st):
            return lambda: emit_v_mtile(xv_c[st // 4], (st // 4) * 512, st)

        def stage_thunk(lst, xdr, c, tag, name):
            return lambda: lst.append(stage_x(xdr, c, tag, name))

        def k1_chunk(c):
            return lambda: emit_qk_chunk(
                xk_c[4 + c], c * 512, wk_sb, kb_sb, kT_sb, 1, c * 512,
                "gpsimd")

        def q1_chunk(dt, c):
            return lambda: emit_qk_chunk(
                xq_c[c], c * 512, wq_sb, qb_sb, qT_sb, dt, c * 512,
                "gpsimd")  # xq_c[2]/[3] appended by stage thunks

        # filler map: fillers[n][slot] = list of thunks
        fillers = [dict() for _ in range(8)]

        def add(n, slot, thunk):
            fillers[n].setdefault(slot, []).append(thunk)

        # n0: V tiles 0..9 (xv c2/c3 staged mid-stream)
        for st in range(10):
            add(0, st, v_tile(st))
        add(0, 6, stage_thunk(xv_c, xv, 2, "xv", "xv_c2"))
        add(0, 10, stage_thunk(xv_c, xv, 3, "xv", "xv_c3"))
        # n1: V tiles 10..15 early
        for st in range(10, 16):
            add(1, st - 10, v_tile(st))
        # K-t1 gates scores of n2 (h2 reads kT tile 1): chunks c0/c1 land
        # in n0; c2/c3 in n2 slots 0..4 (only kts>=8 read their columns).
        add(0, 10, stage_thunk(xk_c, xk, 0, "xk", "xk_c0b"))
        add(0, 12, k1_chunk(0))
        add(0, 12, stage_thunk(xk_c, xk, 1, "xk", "xk_c1b"))
        add(0, 14, k1_chunk(1))
        add(2, 0, stage_thunk(xk_c, xk, 2, "xk", "xk_c2b"))
        add(2, 2, k1_chunk(2))
        add(2, 2, stage_thunk(xk_c, xk, 3, "xk", "xk_c3b"))
        add(2, 4, k1_chunk(3))
        # n3: all Q-qc1 chunks (gate n4 scores)
        add(3, 2, stage_thunk(xq_c, xq, 2, "xq", "xq_c2"))
        add(3, 6, stage_thunk(xq_c, xq, 3, "xq", "xq_c3"))
        add(3, 8, q1_chunk(0, 2))
        add(3, 10, q1_chunk(0, 3))
        add(3, 12, q1_chunk(1, 2))
        add(3, 14, q1_chunk(1, 3))
        # n4/n5: outproj qc0 tiles (norm of n3 lands at n4 slot 8)
        for i, st in enumerate(range(0, 4)):
            add(4, 9 + 2 * i, lambda st=st: emit_outproj(st))
        for i, st in enumerate(range(4, 8)):
            add(5, 9 + 2 * i, lambda st=st: emit_outproj(st))

        prev = None  # (qc, h, pts, o_ps)
        for n, (qc, h) in enumerate(seq):
            o_ps = opspool.tile([HD + 1, QW], f32, tag="ops",
                                name=f"ops_{qc}_{h}")
            pts = []
            for kt in range(KT):
                for thunk in fillers[n].get(kt, ()):
                    thunk()
                if prev is not None and kt < 8:
                    pqc, ph, ppts, po_ps = prev
                    for j in (2 * kt, 2 * kt + 1):
                        emit_pv_kt(ph, j, ppts[j], po_ps)
                pts.append(emit_scores_kt(qc, h, kt))
                if prev is not None and kt == 8:
                    pqc, ph, ppts, po_ps = prev
                    posb, prec = emit_osb_copy(pqc, ph, po_ps)
                    emit_norm(pqc, ph, posb, prec)
            prev = (qc, h, pts, o_ps)

        # tail: PV + norm of the last head, then outproj qc1
        pqc, ph, ppts, po_ps = prev
        for kt in range(KT):
            emit_pv_kt(ph, kt, ppts[kt], po_ps)
        posb, prec = emit_osb_copy(pqc, ph, po_ps)
        emit_norm(pqc, ph, posb, prec)
        for st in range(8, 16):
            emit_outproj(st)

    nc.compile()
    return nc


@functools.lru_cache(maxsize=1)
def _graph():
    return build_graph()


def make_in_maps(
    query, key, value, mask,
    wq_kernel, wq_bias, wk_kernel, wk_bias,
    wv_kernel, wv_bias, wo_kernel, wo_bias,
):
    q = np.asarray(query, np.float32)
    k = np.asarray(key, np.float32)
    v = np.asarray(value, np.float32)
    mask = np.asarray(mask)
    wqk = np.asarray(wq_kernel, np.float32)
    wkk = np.asarray(wk_kernel, np.float32)
    wvk = np.asarray(wv_kernel, np.float32)
    wok = np.asarray(wo_kernel, np.float32)

    def tile_x(a):  # [S, D] -> [P, TDIN, S] pre-tiled transpose
        return np.ascontiguousarray(
            a.T.reshape(TDIN, P, S).transpose(1, 0, 2)
        ).astype(BF16)

    xt = [[tile_x(x[b]) for x in (q, k, v)] for b in range(B)]
    mt = [
        np.ascontiguousarray(mask[b].T.astype(np.float32)).astype(BF16)
        for b in range(B)
    ]
    in_maps = []
    for c in range(NCORES):
        b, g = divmod(c, GH)
        cs = slice(g * DG, (g + 1) * DG)
        # wo rows for this group: [256, D] -> [128, NT2, D] with partition
        # p = (h%2)*64+hd, tile t = h//2  (head pair packing).
        wog = wok[cs, :].reshape(HPG, HD, D)        # [h, hd, n]
        wo_arr = np.ascontiguousarray(
            wog.reshape(NT2, 2, HD, D)               # [t, h%2, hd, n]
            .transpose(1, 2, 0, 3)                   # [h%2, hd, t, n]
            .reshape(P, NT2, D)
        ).astype(BF16)
        # q/k biases as [128, NT2] per-partition columns (dout tiles)
        qb_arr = np.ascontiguousarray(
            np.asarray(wq_bias, np.float32)[cs].reshape(NT2, P).T
        )
        kb_arr = np.ascontiguousarray(
            np.asarray(wk_bias, np.float32)[cs].reshape(NT2, P).T
        )
        in_maps.append(
            {
                "xq_t": xt[b][0],
                "xk_t": xt[b][1],
                "xv_t": xt[b][2],
                "mask_t": mt[b],
                "wq": np.ascontiguousarray(wqk[:, cs].reshape(TDIN, P, DG).transpose(1, 0, 2)).astype(BF16),
                "wk": np.ascontiguousarray(wkk[:, cs].reshape(TDIN, P, DG).transpose(1, 0, 2)).astype(BF16),
                "wv": np.ascontiguousarray(wvk[:, cs].reshape(TDIN, P, DG).transpose(1, 0, 2)).astype(BF16),
                "wo": wo_arr,
                "qb": qb_arr,
                "kb": kb_arr,
                "vb": np.asarray(wv_bias, np.float32)[cs].reshape(1, DG).astype(BF16),
            }
        )
    return in_maps


def combine_outputs(results, wo_bias):
    outs = np.stack([np.asarray(r["out"], np.float32) for r in results])
    full = outs.reshape(B, GH, S, D).sum(axis=1)
    return (full + np.asarray(wo_bias, np.float32)[None, None, :]).astype(
        np.float32
    )


def kernel(**inputs):
    from concourse import bass_utils

    nc = _graph()
    in_maps = make_in_maps(**inputs)
    res = bass_utils.run_bass_kernel_spmd(
        nc, in_maps, core_ids=list(range(NCORES))
    )
    return combine_outputs(res.results, inputs["wo_bias"])


# revision 18
# speedup vs baseline: 1.1857x; 1.0490x over previous
# Multi-head attention (B=2, S=2048, D=1024, H=16) on 8 TRN2 NeuronCores.
#
# Sharding (hardcoded): core c in [0..8) handles batch b = c//4 and head
# group g = c%4 (4 heads = 256 output features of wq/wk/wv, 256 input rows
# of wo). Each core computes a partial output projection [S, D]; the host
# sums the 4 partials per batch and adds wo_bias (row-parallel unshard).
#
# v2 design notes (engines balanced around the Scalar exp floor, ~147us):
#   - all matmuls bf16 (fp8 fails the 2e-2 tolerance: random-sign dots keep
#     the per-element quant error, and scores enter exp).
#   - activations enter transposed ([D, S]); scores computed transposed
#     (S^T[k, q]) so softmax(P) feeds P@V directly; denominator comes free
#     as a ones-column appended to each head's V block.
#   - output projection contracts K=128 by packing head PAIRS into
#     otn2[128, t, S]; odd heads reach partitions 64..127 via a small
#     SBUF->SBUF DMA (engines cannot shift partitions).
#   - emission is pipelined so exp starts ~12us in: Q-sc0 -> K-t0 ->
#     scores-h0 ... with V-projection tiles and K-t1 interleaved between
#     score tiles to fill the PE while exp runs. h0's PV is deferred (its
#     P tiles stay resident) until V exists; o_ps is copied to SBUF right
#     after PV so the single PSUM accumulator frees quickly.
#   - Q/K biases fused into drains (scalar Identity+bias during idle proj
#     phase); late drains and outproj drains go to GpSimd so the Vector
#     engine mostly runs mask-mult + normalization.
import functools
import sys

import numpy as np

try:
    import concourse  # noqa: F401
except ImportError:  # harness env without the default path
    sys.path.insert(0, "/opt/trn_rl_repo")
    sys.path.insert(0, "/opt/pypackages")

import ml_dtypes

BF16 = ml_dtypes.bfloat16

B, S, D, H = 2, 2048, 1024, 16
HD = D // H          # 64
NCORES = 8
GH = 4               # head groups (tensor-parallel)
HPG = H // GH        # heads per group = 4
DG = D // GH         # features per group = 256
P = 128              # partitions
TDIN = D // P        # 8 din tiles
QC = 2               # q-chunks of 1024 for attention
QW = S // QC         # 1024
KT = S // P          # 16 k tiles
NT2 = DG // P        # 2 dout tiles per group


def build_graph():
    """Build the SPMD Bass graph (identical on all 8 cores)."""
    from contextlib import ExitStack

    from concourse import bacc, mybir, tile

    f32 = mybir.dt.float32
    bf16 = mybir.dt.bfloat16
    EXP = mybir.ActivationFunctionType.Exp
    IDENT = mybir.ActivationFunctionType.Identity

    nc = bacc.Bacc(
        "TRN2", target_bir_lowering=False, debug=False, num_devices=NCORES
    )

    xq = nc.dram_tensor("xq_t", (P, TDIN, S), bf16, kind="ExternalInput")
    xk = nc.dram_tensor("xk_t", (P, TDIN, S), bf16, kind="ExternalInput")
    xv = nc.dram_tensor("xv_t", (P, TDIN, S), bf16, kind="ExternalInput")
    mk = nc.dram_tensor("mask_t", (S, S), bf16, kind="ExternalInput")
    wq = nc.dram_tensor("wq", (P, TDIN, DG), bf16, kind="ExternalInput")
    wk = nc.dram_tensor("wk", (P, TDIN, DG), bf16, kind="ExternalInput")
    wv = nc.dram_tensor("wv", (P, TDIN, DG), bf16, kind="ExternalInput")
    # wo pre-arranged host-side to [128, NT2, D]: partition p = (h%2)*64+hd,
    # tile t = h//2 (head pair), so outproj contracts K=128 over 2 heads.
    wo = nc.dram_tensor("wo", (P, NT2, D), bf16, kind="ExternalInput")
    # q/k biases as per-partition columns [128, NT2]; v bias as a row.
    qb = nc.dram_tensor("qb", (P, NT2), f32, kind="ExternalInput")
    kb = nc.dram_tensor("kb", (P, NT2), f32, kind="ExternalInput")
    vb = nc.dram_tensor("vb", (1, DG), bf16, kind="ExternalInput")
    out = nc.dram_tensor("out", (S, D), bf16, kind="ExternalOutput")

    with tile.TileContext(nc) as tc, ExitStack() as ctx:
        wpool = ctx.enter_context(tc.tile_pool(name="wpool", bufs=1))
        cpool = ctx.enter_context(tc.tile_pool(name="cpool", bufs=1))
        qkpool = ctx.enter_context(tc.tile_pool(name="qk", bufs=1))
        vpool = ctx.enter_context(tc.tile_pool(name="vsb", bufs=1))
        mpool = ctx.enter_context(tc.tile_pool(name="msk", bufs=1))
        opool = ctx.enter_context(tc.tile_pool(name="otn", bufs=1))
        xstage = ctx.enter_context(tc.tile_pool(name="xin", bufs=2))
        ptpool = ctx.enter_context(tc.tile_pool(name="pt", bufs=16))
        npool = ctx.enter_context(tc.tile_pool(name="nrm", bufs=1))
        ospool = ctx.enter_context(tc.tile_pool(name="osbp", bufs=1))
        dpool = ctx.enter_context(tc.tile_pool(name="dscr", bufs=2, space="DRAM"))
        # PSUM: scores [128,1024] x2 (4 banks) + o_ps [65,1024] x1 (2 banks)
        # + proj/outproj [128,512] x2 (2 banks) = 8 banks exactly.
        spspool = ctx.enter_context(tc.tile_pool(name="sps", bufs=2, space="PSUM"))
        opspool = ctx.enter_context(tc.tile_pool(name="ops", bufs=1, space="PSUM"))
        pjpool = ctx.enter_context(tc.tile_pool(name="pjps", bufs=2, space="PSUM"))

        # ---- persistent SBUF tensors & input DMAs ------------------------
        # Per-tensor staging pools (512-col chunks) so no staging DMA ever
        # waits on a reader that sits later in the PE stream (deadlock).
        def stage_x(xdr, c, tag, name):
            xt = xstage.tile([P, TDIN, 512], bf16, tag=tag, name=name)
            for th_ in range(4):
                nc.sync.dma_start(
                    xt[:, th_ * 2 : (th_ + 1) * 2, :],
                    xdr.ap()[:, th_ * 2 : (th_ + 1) * 2, c * 512 : (c + 1) * 512],
                )
            return xt

        xq_c = [stage_x(xq, 0, "xq", "xq_c0"), stage_x(xq, 1, "xq", "xq_c1")]
        wq_sb = wpool.tile([P, TDIN, DG], bf16)
        wk_sb = wpool.tile([P, TDIN, DG], bf16)
        for wsb_, wdr_ in ((wq_sb, wq), (wk_sb, wk)):
            for th_ in range(2):
                nc.sync.dma_start(
                    wsb_[:, th_ * 4 : (th_ + 1) * 4, :],
                    wdr_.ap()[:, th_ * 4 : (th_ + 1) * 4, :],
                )
        qb_sb = cpool.tile([P, NT2], f32)
        kb_sb = cpool.tile([P, NT2], f32)
        nc.sync.dma_start(qb_sb[:], qb.ap())
        nc.sync.dma_start(kb_sb[:], kb.ap())

        xk_c = [stage_x(xk, 0, "xk", "xk_c0"), stage_x(xk, 1, "xk", "xk_c1")]

        # mask per-kt: first 6 upfront, rest staggered into n0's slots so
        # they don't clog the DMA queues ahead of the staging transfers.
        mask_sb = mpool.tile([P, KT, S], bf16)
        mk_r = mk.ap().rearrange("(t p) q -> p t q", p=P)
        for kt in range(6):
            nc.sync.dma_start(mask_sb[:, kt, :], mk_r[:, kt, :])

        def mask_dma(kt):
            return lambda: nc.sync.dma_start(mask_sb[:, kt, :], mk_r[:, kt, :])

        wv_sb = wpool.tile([P, TDIN, DG], bf16)
        for th_ in range(2):
            nc.sync.dma_start(
                wv_sb[:, th_ * 4 : (th_ + 1) * 4, :],
                wv.ap()[:, th_ * 4 : (th_ + 1) * 4, :],
            )
        wo_sb = wpool.tile([P, NT2, D], bf16)
        nc.sync.dma_start(wo_sb[:], wo.ap())
        vb_sb = cpool.tile([1, DG], bf16)
        nc.sync.dma_start(vb_sb[:], vb.ap())
        ones2 = cpool.tile([1, P], bf16)
        nc.vector.memset(ones2[:], 1.0)

        qT_sb = qkpool.tile([P, NT2, S], bf16)   # q projection, transposed
        kT_sb = qkpool.tile([P, NT2, S], bf16)
        # v blocks: per k-tile, per head: [v(64) | ones] -> 65 cols
        v_sb = vpool.tile([P, KT, HPG * (HD + 1)], bf16)
        nc.vector.memset(
            v_sb[:].rearrange("p s (h x) -> p s h x", h=HPG)[:, :, :, HD : HD + 1],
            1.0,
        )
        # packed normalized attention output: partition (h%2)*64+hd, tile h//2
        otn2 = opool.tile([P, NT2, S], bf16)

        # ---- projection helpers ------------------------------------------
        def emit_qk_chunk(xsb, xoff, wsb, bias_sb, dest, dt, s0, drain_eng):
            """Project one [128 dout, 512 s] tile: 8 acc matmuls + drain."""
            ps = pjpool.tile([P, 512], f32, tag="pj", name=f"pj_{dt}_{s0}_{drain_eng}")
            for ktl in range(TDIN):
                nc.tensor.matmul(
                    ps[:],
                    lhsT=wsb[:, ktl, dt * P : (dt + 1) * P],
                    rhs=xsb[:, ktl, s0 - xoff : s0 - xoff + 512],
                    start=(ktl == 0),
                    stop=(ktl == TDIN - 1),
                )
            if drain_eng == "scalar":
                nc.scalar.activation(
                    dest[:, dt, s0 : s0 + 512], ps[:], IDENT,
                    bias=bias_sb[:, dt : dt + 1], scale=1.0,
                )
            else:
                nc.vector.tensor_scalar_add(
                    dest[:, dt, s0 : s0 + 512], ps[:], bias_sb[:, dt : dt + 1]
                )

        def emit_v_mtile(xsb, xoff, st):
            """V projection for s-tile st ([128 s, 256 dout] + bias + ones)."""
            ps = pjpool.tile([P, 512], f32, tag="pj", name=f"pv_{st}")
            for ktl in range(TDIN):
                nc.tensor.matmul(
                    ps[:, 0:DG],
                    lhsT=xsb[:, ktl, st * P - xoff : (st + 1) * P - xoff],
                    rhs=wv_sb[:, ktl, :],
                    start=(ktl == 0),
                    stop=False,
                )
            nc.tensor.matmul(
                ps[:, 0:DG],
                lhsT=ones2[0:1, :],
                rhs=vb_sb[:],
                start=False,
                stop=True,
            )
            dst = v_sb[:, st, :].rearrange("p (h x) -> p h x", h=HPG)[:, :, 0:HD]
            src = ps[:, 0:DG].rearrange("p (h x) -> p h x", h=HPG)
            nc.vector.tensor_copy(dst, src)

        # ---- attention helpers -------------------------------------------
        def emit_scores_kt(qc, h, kt):
            """scores (2 matmuls) -> exp -> mask-mult; returns P tile."""
            t, po = h // 2, (h % 2) * HD
            s_ps = spspool.tile([P, QW], f32, tag="sps", name=f"sps_{qc}_{h}_{kt}")
            for hf in range(2):
                nc.tensor.matmul(
                    s_ps[:, hf * 512 : (hf + 1) * 512],
                    lhsT=kT_sb[po : po + HD, t, kt * P : (kt + 1) * P],
                    rhs=qT_sb[
                        po : po + HD, t,
                        qc * QW + hf * 512 : qc * QW + (hf + 1) * 512,
                    ],
                    start=True,
                    stop=True,
                )
            pt = ptpool.tile([P, QW], bf16, tag="p", name=f"pt_{qc}_{h}_{kt}")
            nc.scalar.activation(pt[:], s_ps[:], EXP, scale=0.125)
            meng = nc.gpsimd if kt in (4, 9, 14) else nc.vector
            meng.tensor_mul(
                pt[:], pt[:], mask_sb[:, kt, qc * QW : (qc + 1) * QW]
            )
            return pt

        def emit_pv_kt(h, kt, pt, o_ps):
            for hf in range(2):
                nc.tensor.matmul(
                    o_ps[:, hf * 512 : (hf + 1) * 512],
                    lhsT=v_sb[:, kt, h * 65 : (h + 1) * 65],
                    rhs=pt[:, hf * 512 : (hf + 1) * 512],
                    start=(kt == 0),
                    stop=(kt == KT - 1),
                )

        def emit_osb_copy(qc, h, o_ps):
            """Reciprocal (needs f32) + SBUF copy so the PSUM frees quickly."""
            rec65 = npool.tile([HD + 1, QW], f32, tag="rec")
            nc.vector.reciprocal_approx_fast(out=rec65[:], in_=o_ps[:])
            osb = ospool.tile([HD + 1, QW], bf16, tag="osb", name=f"osb_{qc}_{h}")
            nc.vector.tensor_copy(osb[:], o_ps[:])
            return osb, rec65

        def emit_norm(qc, h, osb, rec65):
            """softmax normalization; writes otn2 (odd heads via DMA)."""
            t = h // 2
            scr = dpool.tile([1, QW], f32, tag="scr", name=f"scr_{qc}_{h}")
            nc.sync.dma_start(scr[:], rec65[HD : HD + 1, :])
            rb = npool.tile([HD, QW], f32, tag="rb")
            nc.sync.dma_start(rb[:], scr[:].to_broadcast((HD, QW)))
            if h % 2 == 0:
                nc.vector.tensor_mul(
                    otn2[0:HD, t, qc * QW : (qc + 1) * QW], osb[0:HD, :], rb[:]
                )
            else:
                nc.vector.tensor_mul(osb[0:HD, :], osb[0:HD, :], rb[:])
                nc.sync.dma_start(
                    otn2[HD:P, t, qc * QW : (qc + 1) * QW], osb[0:HD, :]
                )

        def emit_outproj(st, drain="vector"):
            osb2 = ospool.tile([P, D], bf16, tag="outsb", name=f"outsb_{st}",
                               bufs=2)
            for nch in range(2):
                op_ps = pjpool.tile(
                    [P, 512], f32, tag="pj", name=f"ops2_{st}_{nch}"
                )
                for t in range(NT2):
                    nc.tensor.matmul(
                        op_ps[:],
                        lhsT=otn2[:, t, st * P : (st + 1) * P],
                        rhs=wo_sb[:, t, nch * 512 : (nch + 1) * 512],
                        start=(t == 0),
                        stop=(t == NT2 - 1),
                    )
                dst = osb2[:, nch * 512 : (nch + 1) * 512]
                if drain == "scalar":
                    nc.scalar.copy(dst, op_ps[:])
                else:
                    nc.vector.tensor_copy(dst, op_ps[:])
            nc.sync.dma_start(out.ap()[st * P : (st + 1) * P, :], osb2[:])

        # ---- emission schedule -------------------------------------------
        # Uniform deferred-by-one pipeline: head-sequence n = (qc, h); each
        # head's 16 score-kt "slots" carry fillers: PV of the PREVIOUS head
        # (2 kt-pairs per slot over slots 0-7), V-projection tiles, K-t1 /
        # Q-sc1 chunks (re-staged with prefetch), and outproj tiles.
        seq = [(qc, h) for qc in range(QC) for h in range(HPG)]

        # prefix: all Q-qc0 + K-t0 chunks (xk c2/c3 staged rolling).
        for dt in range(NT2):
            for c in range(2):
                emit_qk_chunk(xq_c[c], c * 512, wq_sb, qb_sb, qT_sb, dt,
                              c * 512, "scalar")
        for c in range(4):
            if c >= 2:
                xk_c.append(stage_x(xk, c, "xk", f"xk_c{c}"))
            emit_qk_chunk(xk_c[c], c * 512, wk_sb, kb_sb, kT_sb, 0,
                          c * 512, "scalar")

        xv_c = [stage_x(xv, 0, "xv", "xv_c0"), stage_x(xv, 1, "xv", "xv_c1")]

        def v_tile(st):
            return lambda: emit_v_mtile(xv_c[st // 4], (st // 4) * 512, st)

        def stage_thunk(lst, xdr, c, tag, name):
            return lambda: lst.append(stage_x(xdr, c, tag, name))

        def k1_chunk(c):
            return lambda: emit_qk_chunk(
                xk_c[4 + c], c * 512, wk_sb, kb_sb, kT_sb, 1, c * 512,
                "gpsimd")

        def q1_chunk(dt, c):
            return lambda: emit_qk_chunk(
                xq_c[c], c * 512, wq_sb, qb_sb, qT_sb, dt, c * 512,
                "gpsimd")  # xq_c[2]/[3] appended by stage thunks

        # filler map: fillers[n][slot] = list of thunks
        fillers = [dict() for _ in range(8)]

        def add(n, slot, thunk):
            fillers[n].setdefault(slot, []).append(thunk)

        # n0: V tiles 0..9 (xv c2/c3 staged mid-stream) + staggered mask DMAs
        for st in range(10):
            add(0, st, v_tile(st))
        for kt in range(6, KT):
            add(0, kt - 6, mask_dma(kt))
        add(0, 4, stage_thunk(xv_c, xv, 2, "xv", "xv_c2"))
        add(0, 7, stage_thunk(xv_c, xv, 3, "xv", "xv_c3"))
        # n1: V tiles 10..15 early
        for st in range(10, 16):
            add(1, st - 10, v_tile(st))
        # K-t1 gates scores of n2 (h2 reads kT tile 1): chunks c0/c1 land
        # in n0; c2/c3 in n2 slots 0..4 (only kts>=8 read their columns).
        add(0, 6, stage_thunk(xk_c, xk, 0, "xk", "xk_c0b"))
        add(0, 10, k1_chunk(0))
        add(0, 8, stage_thunk(xk_c, xk, 1, "xk", "xk_c1b"))
        add(0, 12, k1_chunk(1))
        add(2, 0, stage_thunk(xk_c, xk, 2, "xk", "xk_c2b"))
        add(2, 2, k1_chunk(2))
        add(2, 2, stage_thunk(xk_c, xk, 3, "xk", "xk_c3b"))
        add(2, 4, k1_chunk(3))
        # n3: all Q-qc1 chunks (gate n4 scores)
        add(3, 2, stage_thunk(xq_c, xq, 2, "xq", "xq_c2"))
        add(3, 6, stage_thunk(xq_c, xq, 3, "xq", "xq_c3"))
        add(3, 8, q1_chunk(0, 2))
        add(3, 10, q1_chunk(0, 3))
        add(3, 12, q1_chunk(1, 2))
        add(3, 14, q1_chunk(1, 3))
        # n4/n5: outproj qc0 tiles (norm of n3 lands at n4 slot 8)
        for i, st in enumerate(range(0, 4)):
            add(4, 9 + 2 * i, lambda st=st: emit_outproj(st))
        for i, st in enumerate(range(4, 8)):
            add(5, 9 + 2 * i, lambda st=st: emit_outproj(st))

        prev = None  # (qc, h, pts, o_ps)
        for n, (qc, h) in enumerate(seq):
            o_ps = opspool.tile([HD + 1, QW], f32, tag="ops",
                                name=f"ops_{qc}_{h}")
            pts = []
            for kt in range(KT):
                for thunk in fillers[n].get(kt, ()):
                    thunk()
                if prev is not None and kt < 8:
                    pqc, ph, ppts, po_ps = prev
                    for j in (2 * kt, 2 * kt + 1):
                        emit_pv_kt(ph, j, ppts[j], po_ps)
                pts.append(emit_scores_kt(qc, h, kt))
                if prev is not None and kt == 8:
                    pqc, ph, ppts, po_ps = prev
                    posb, prec = emit_osb_copy(pqc, ph, po_ps)
                    emit_norm(pqc, ph, posb, prec)
                # last head: inline its PV once the accumulator is free
                if n == len(seq) - 1 and kt >= 9:
                    for j in (2 * (kt - 9), 2 * (kt - 9) + 1):
                        emit_pv_kt(h, j, pts[j], o_ps)
            prev = (qc, h, pts, o_ps)

        # tail: finish last head's PV + norm, then outproj qc1 with
        # Scalar drains (exp is done by now).
        pqc, ph, ppts, po_ps = prev
        for kt in range(14, KT):
            emit_pv_kt(ph, kt, ppts[kt], po_ps)
        posb, prec = emit_osb_copy(pqc, ph, po_ps)
        emit_norm(pqc, ph, posb, prec)
        for st in range(8, 16):
            emit_outproj(st, drain="scalar")

    nc.compile()
    return nc


@functools.lru_cache(maxsize=1)
def _graph():
    return build_graph()


def make_in_maps(
    query, key, value, mask,
    wq_kernel, wq_bias, wk_kernel, wk_bias,
    wv_kernel, wv_bias, wo_kernel, wo_bias,
):
    q = np.asarray(query, np.float32)
    k = np.asarray(key, np.float32)
    v = np.asarray(value, np.float32)
    mask = np.asarray(mask)
    wqk = np.asarray(wq_kernel, np.float32)
    wkk = np.asarray(wk_kernel, np.float32)
    wvk = np.asarray(wv_kernel, np.float32)
    wok = np.asarray(wo_kernel, np.float32)

    def tile_x(a):  # [S, D] -> [P, TDIN, S] pre-tiled transpose
        return np.ascontiguousarray(
            a.T.reshape(TDIN, P, S).transpose(1, 0, 2)
        ).astype(BF16)

    xt = [[tile_x(x[b]) for x in (q, k, v)] for b in range(B)]
    mt = [
        np.ascontiguousarray(mask[b].T.astype(np.float32)).astype(BF16)
        for b in range(B)
    ]
    in_maps = []
    for c in range(NCORES):
        b, g = divmod(c, GH)
        cs = slice(g * DG, (g + 1) * DG)
        # wo rows for this group: [256, D] -> [128, NT2, D] with partition
        # p = (h%2)*64+hd, tile t = h//2  (head pair packing).
        wog = wok[cs, :].reshape(HPG, HD, D)        # [h, hd, n]
        wo_arr = np.ascontiguousarray(
            wog.reshape(NT2, 2, HD, D)               # [t, h%2, hd, n]
            .transpose(1, 2, 0, 3)                   # [h%2, hd, t, n]
            .reshape(P, NT2, D)
        ).astype(BF16)
        # q/k biases as [128, NT2] per-partition columns (dout tiles)
        qb_arr = np.ascontiguousarray(
            np.asarray(wq_bias, np.float32)[cs].reshape(NT2, P).T
        )
        kb_arr = np.ascontiguousarray(
            np.asarray(wk_bias, np.float32)[cs].reshape(NT2, P).T
        )
        in_maps.append(
            {
                "xq_t": xt[b][0],
                "xk_t": xt[b][1],
                "xv_t": xt[b][2],
                "mask_t": mt[b],
                "wq": np.ascontiguousarray(wqk[:, cs].reshape(TDIN, P, DG).transpose(1, 0, 2)).astype(BF16),
                "wk": np.ascontiguousarray(wkk[:, cs].reshape(TDIN, P, DG).transpose(1, 0, 2)).astype(BF16),
                "wv": np.ascontiguousarray(wvk[:, cs].reshape(TDIN, P, DG).transpose(1, 0, 2)).astype(BF16),
                "wo": wo_arr,
                "qb": qb_arr,
                "kb": kb_arr,
                "vb": np.asarray(wv_bias, np.float32)[cs].reshape(1, DG).astype(BF16),
            }
        )
    return in_maps


def combine_outputs(results, wo_bias):
    outs = np.stack([np.asarray(r["out"], np.float32) for r in results])
    full = outs.reshape(B, GH, S, D).sum(axis=1)
    return (full + np.asarray(wo_bias, np.float32)[None, None, :]).astype(
        np.float32
    )


def kernel(**inputs):
    from concourse import bass_utils

    nc = _graph()
    in_maps = make_in_maps(**inputs)
    res = bass_utils.run_bass_kernel_spmd(
        nc, in_maps, core_ids=list(range(NCORES))
    )
    return combine_outputs(res.results, inputs["wo_bias"])
